# revision 1
# baseline (speedup 1.0000x reference)
"""CopyGenerator kernel for 8 Trainium2 cores.

Sharding: batch-parallel attention (core c owns batch c) + vocab-parallel
out_fc/scatter (core c owns extended-vocab slice [c*4016,(c+1)*4016)).
One AllGather mid-kernel moves the tiny per-batch attention results
(layernormed+gated att, gate p, partial layernorm stats) to every core.
"""

import numpy as np
import os
_STAGE = int(os.environ.get("K_STAGE", "3"))

import concourse.bacc as bacc
import concourse.bass as bass
import concourse.tile as tile
from concourse import mybir
from concourse.bass_utils import run_bass_kernel_spmd
from concourse.masks import make_identity

B, TQ, TK, D, V = 8, 64, 128, 512, 32000
EXT = V + TK            # 32128
NCORE = 8
VS = EXT // NCORE       # 4016 ext columns per core
NE, EC = 8, 502         # e-chunks per core slice
DC = D // 128           # 4 contraction chunks
NT = (B * TQ) // 128    # 4 token tiles of 128
QBLK = 4                # queries per energy block
NQB = TQ // QBLK        # 16 blocks
P = 128

F32 = mybir.dt.float32
F32R = mybir.dt.float32r
I32 = mybir.dt.int32
OP = mybir.AluOpType
AX = mybir.AxisListType
ACT = mybir.ActivationFunctionType

_CACHE = {}


def _bc(ap, parts):
    """Partition-stride-0 broadcast AP (DMA/matmul sources only)."""
    return bass.AP(tensor=ap.tensor, offset=ap.offset,
                   ap=[[0, parts]] + [list(p) for p in ap.ap])


def _r(ap):
    return ap.bitcast(F32R)


def _build(has_bout):
    nc = bacc.Bacc("TRN2", target_bir_lowering=False, debug=False,
                   num_devices=NCORE)
    io = {}
    def din(name, shape, dt=F32):
        io[name] = nc.dram_tensor(name, shape, dt, kind="ExternalInput")
    din("wT", [D, VS], F32R)          # W_pad.T slice
    din("tgtT", [D, B * TQ], F32R)    # all tokens, transposed
    din("tgtTo", [D, TQ], F32R)       # own batch tokens, transposed
    din("tgto", [TQ, D])        # own batch tokens
    din("skey", [TK, D], F32R)
    din("skeyT", [D, TK], F32R)
    din("wqT", [D, D], F32R)
    din("wkT", [D, D], F32R)
    din("battn", [1, D], F32R)
    din("vw", [1, D])
    din("wlin", [1, D])
    din("blin", [1, 1])
    din("idx", [B, TK])         # src_map_idx - c*VS, as f32
    din("colmask", [1, P])
    din("onesrow", [1, P], F32R)
    if has_bout:
        din("bvec", [1, VS], F32R)
    out = nc.dram_tensor("out", [B * TQ, VS], F32, kind="ExternalOutput")

    with tile.TileContext(nc) as tc:
        _emit(nc, tc, io, out, has_bout)
    nc.compile()
    return nc


def _emit(nc, tc, io, out, has_bout):
    from contextlib import ExitStack
    ctx = ExitStack()
    with ctx:
        sing = ctx.enter_context(tc.tile_pool(name="sing", bufs=1))
        dram = ctx.enter_context(tc.tile_pool(name="dram", bufs=1, space="DRAM"))
        ps_a = ctx.enter_context(tc.tile_pool(name="psa", bufs=1, space="PSUM"))
        ps_b = ctx.enter_context(tc.tile_pool(name="psb", bufs=2, space="PSUM"))
        ps_d = ctx.enter_context(tc.tile_pool(name="psd", bufs=2, space="PSUM"))

        # ---------------- persistent loads ----------------
        wt = [sing.tile([P, VS], F32R, tag=f"wt{d}", name=f"wt{d}") for d in range(DC)]
        for d in range(DC):
            nc.sync.dma_start(out=wt[d][:], in_=io["wT"][d * P:(d + 1) * P, :])
        tgtT = [sing.tile([P, B * TQ], F32R, tag=f"tgtT{d}", name=f"tgtT{d}") for d in range(DC)]
        for d in range(DC):
            nc.sync.dma_start(out=tgtT[d][:], in_=io["tgtT"][d * P:(d + 1) * P, :])
        skey = sing.tile([TK, D], F32R)
        nc.sync.dma_start(out=skey[:], in_=io["skey"][:, :])
        tgto = sing.tile([TQ, D], F32)
        nc.sync.dma_start(out=tgto[:], in_=io["tgto"][:, :])
        vw_rep = sing.tile([P, D], F32)
        nc.gpsimd.dma_start(out=vw_rep[:], in_=_bc(io["vw"][0:1, :], P))
        wlin_rep = sing.tile([TQ, D], F32)
        nc.gpsimd.dma_start(out=wlin_rep[:], in_=_bc(io["wlin"][0:1, :], TQ))
        blin64 = sing.tile([TQ, 1], F32)
        nc.gpsimd.dma_start(out=blin64[:], in_=_bc(io["blin"][0:1, :], TQ))
        cmask_rep = sing.tile([P, P], F32)
        nc.gpsimd.dma_start(out=cmask_rep[:], in_=_bc(io["colmask"][0:1, :], P))
        idxT = sing.tile([TK, B], F32)
        nc.sync.dma_start(out=idxT[:], in_=io["idx"][:, :].rearrange("b k -> k b"))
        if has_bout:
            bvec = sing.tile([1, VS], F32R)
            nc.sync.dma_start(out=bvec[:], in_=io["bvec"][:, :])
        ones1p = sing.tile([1, P], F32R)
        nc.sync.dma_start(out=ones1p[:], in_=io["onesrow"][:, :])
        ident = sing.tile([P, P], F32)
        make_identity(nc, ident[:])
        eps64 = sing.tile([TQ, 1], F32)
        nc.vector.memset(eps64[:], 1e-5)
        eps128 = sing.tile([P, 1], F32)
        nc.vector.memset(eps128[:], 1e-5)

        logits = [sing.tile([P, VS], F32, tag=f"log{t}", name=f"log{t}") for t in range(NT)]
        pack = sing.tile([P, 73], F32R)
        gath = sing.tile([P, NCORE, 73], F32R)
        p_all = sing.tile([P, NT], F32R)

        qp_dram = dram.tile([TQ, D], F32)
        cc_in = dram.tile([P, 73], F32R)
        cc_out = dram.tile([NCORE * P, 73], F32R)

        # ---------------- phase A: attention (own batch) ----------------
        ph1 = ctx.enter_context(tc.tile_pool(name="ph1", bufs=1))

        with tc.tile_pool(name="phw", bufs=1) as phw:
            tgtTo = [phw.tile([P, TQ], F32R, tag=f"tgtTo{d}", name=f"tgtTo{d}")
                     for d in range(DC)]
            wq = [phw.tile([P, D], F32R, tag=f"wq{d}", name=f"wq{d}")
                  for d in range(DC)]
            wk = [phw.tile([P, D], F32R, tag=f"wk{d}", name=f"wk{d}")
                  for d in range(DC)]
            skT = [phw.tile([P, TK], F32R, tag=f"skT{d}", name=f"skT{d}")
                   for d in range(DC)]
            for d in range(DC):
                sl = slice(d * P, (d + 1) * P)
                nc.sync.dma_start(out=tgtTo[d][:], in_=io["tgtTo"][sl, :])
                nc.sync.dma_start(out=wq[d][:], in_=io["wqT"][sl, :])
                nc.sync.dma_start(out=wk[d][:], in_=io["wkT"][sl, :])
                nc.sync.dma_start(out=skT[d][:], in_=io["skeyT"][sl, :])
            battn = phw.tile([1, D], F32R)
            nc.sync.dma_start(out=battn[:], in_=io["battn"][:, :])

            qp_ps = ps_a.tile([TQ, D], F32, space="PSUM", tag="a_qp")
            for d in range(DC):
                nc.tensor.matmul(qp_ps[:], _r(tgtTo[d][:]), _r(wq[d][:]),
                                 start=(d == 0), stop=False)
            nc.tensor.matmul(qp_ps[:], _r(ones1p[:, 0:TQ]), _r(battn[:]),
                             start=False, stop=True)
            qp_sb = ph1.tile([TQ, D], F32)
            nc.scalar.copy(out=qp_sb[:], in_=qp_ps[:])
            nc.sync.dma_start(out=qp_dram[:], in_=qp_sb[:])

            kp_ps = ps_a.tile([TK, D], F32, space="PSUM", tag="a_kp")
            for d in range(DC):
                nc.tensor.matmul(kp_ps[:], _r(skT[d][:]), _r(wk[d][:]),
                                 start=(d == 0), stop=(d == DC - 1))
            kp_sb = ph1.tile([TK, D], F32)
            nc.scalar.copy(out=kp_sb[:], in_=kp_ps[:])


        tmask = ph1.tile([TQ, 1], F32)
        nc.vector.tensor_reduce(out=tmask[:], in_=tgto[:], axis=AX.X, op=OP.add,
                                apply_absolute_value=True)
        nc.scalar.sign(out=tmask[:], in_=tmask[:])
        smask = ph1.tile([TK, 1], F32)
        nc.vector.tensor_reduce(out=smask[:], in_=skey[:], axis=AX.X, op=OP.add,
                                apply_absolute_value=True)
        nc.scalar.sign(out=smask[:], in_=smask[:])

        att_kq = ph1.tile([TK, TQ], F32)
        kp_ap = kp_sb[:]
        kp3 = bass.AP(tensor=kp_ap.tensor, offset=kp_ap.offset,
                      ap=[list(kp_ap.ap[0]), [0, QBLK], list(kp_ap.ap[1])])
        vw_ap = vw_rep[:]
        vw3 = bass.AP(tensor=vw_ap.tensor, offset=vw_ap.offset,
                      ap=[list(vw_ap.ap[0]), [0, QBLK], list(vw_ap.ap[1])])
        with tc.tile_pool(name="pha", bufs=2) as pha:
            for qb in range(NQB):
                qprep = pha.tile([P, QBLK, D], F32, tag="qprep")
                src = qp_dram[qb * QBLK:(qb + 1) * QBLK, :]
                nc.gpsimd.dma_start(out=qprep[:], in_=_bc(src, P))
                et = pha.tile([P, QBLK, D], F32, tag="et")
                nc.vector.tensor_tensor(out=et[:], in0=kp3, in1=qprep[:],
                                        op=OP.add)
                nc.scalar.activation(out=et[:], in_=et[:], func=ACT.Tanh)
                tv = pha.tile([P, QBLK, D], F32, tag="tv")
                nc.vector.tensor_tensor(out=tv[:], in0=et[:], in1=vw3,
                                        op=OP.mult)
                nc.vector.tensor_reduce(
                    out=att_kq[:, qb * QBLK:(qb + 1) * QBLK], in_=tv[:],
                    axis=AX.X, op=OP.add)

        at_ps = ps_a.tile([TQ, P], F32, space="PSUM", tag="a_kp")
        nc.tensor.transpose(out=at_ps[:], in_=att_kq[:], identity=ident[:])
        att_qk = ph1.tile([TQ, P], F32)
        nc.scalar.copy(out=att_qk[:], in_=at_ps[:])

        mrow_ps = ps_a.tile([P, P], F32, space="PSUM", tag="a_qp")
        nc.tensor.transpose(out=mrow_ps[:], in_=smask[:].to_broadcast([P, P]),
                            identity=ident[:])
        pen = ph1.tile([TQ, P], F32)
        nc.vector.tensor_scalar(out=pen[:], in0=mrow_ps[0:TQ, :], scalar1=1.0,
                                scalar2=1e30, op0=OP.subtract, op1=OP.mult)
        attm = ph1.tile([TQ, P], F32)
        nc.vector.tensor_tensor(out=attm[:], in0=att_qk[:],
                                in1=mrow_ps[0:TQ, :], op=OP.mult)
        att_sm = ph1.tile([TQ, P], F32)
        nc.vector.tensor_tensor(out=att_sm[:], in0=attm[:], in1=pen[:], op=OP.add)
        outatt = ph1.tile([TQ, P], F32)
        nc.vector.tensor_scalar(out=outatt[:], in0=attm[:], scalar1=tmask[:],
                                scalar2=None, op0=OP.mult)

        mx = ph1.tile([TQ, 1], F32)
        nc.vector.tensor_reduce(out=mx[:], in_=att_sm[:], axis=AX.X, op=OP.max)
        negmax = ph1.tile([TQ, 1], F32)
        nc.vector.tensor_scalar(out=negmax[:], in0=mx[:], scalar1=-1.0,
                                scalar2=None, op0=OP.mult)
        exps = ph1.tile([TQ, P], F32)
        sumexp = ph1.tile([TQ, 1], F32)
        nc.scalar.activation(out=exps[:], in_=att_sm[:], func=ACT.Exp,
                             bias=negmax[:], scale=1.0, accum_out=sumexp[:])
        rsum = ph1.tile([TQ, 1], F32)
        nc.vector.reciprocal(out=rsum[:], in_=sumexp[:])
        probs = ph1.tile([TQ, P], F32)
        nc.vector.tensor_scalar(out=probs[:], in0=exps[:], scalar1=rsum[:],
                                scalar2=None, op0=OP.mult)

        pt_ps = ps_a.tile([P, TQ], F32, space="PSUM", tag="a_kp")
        nc.tensor.transpose(out=pt_ps[:], in_=probs[:], identity=ident[0:TQ, 0:TQ])
        probsT = ph1.tile([P, TQ], F32R)
        nc.vector.tensor_copy(out=probsT[:], in_=pt_ps[:])
        ctx_ps = ps_a.tile([TQ, D], F32, space="PSUM", tag="a_qp")
        nc.tensor.matmul(ctx_ps[:], _r(probsT[:]), _r(skey[:]),
                         start=True, stop=True)
        scr2 = ph1.tile([TQ, D], F32)
        nc.vector.tensor_tensor(out=scr2[:], in0=ctx_ps[:], in1=wlin_rep[:],
                                op=OP.mult)
        ctxdot = ph1.tile([TQ, 1], F32)
        nc.vector.tensor_reduce(out=ctxdot[:], in_=scr2[:], axis=AX.X,
                                op=OP.add)
        p_q = ph1.tile([TQ, 1], F32)
        nc.scalar.activation(out=p_q[:], in_=ctxdot[:], func=ACT.Sigmoid,
                             bias=blin64[:], scale=tmask[:])
        one_m_p = ph1.tile([TQ, 1], F32)
        nc.vector.tensor_scalar(out=one_m_p[:], in0=p_q[:], scalar1=-1.0,
                                scalar2=1.0, op0=OP.mult, op1=OP.add)

        bst = ph1.tile([TQ, 6], F32)
        nc.vector.bn_stats(out=bst[:], in_=outatt[:])
        mv = ph1.tile([TQ, 2], F32)
        nc.vector.bn_aggr(out=mv[:], in_=bst[:])
        sqv = ph1.tile([TQ, 1], F32)
        nc.scalar.activation(out=sqv[:], in_=mv[:, 1:2], func=ACT.Sqrt, bias=eps64[:])
        rstd_a = ph1.tile([TQ, 1], F32)
        nc.vector.reciprocal(out=rstd_a[:], in_=sqv[:])
        negmean = ph1.tile([TQ, 1], F32)
        nc.vector.tensor_scalar(out=negmean[:], in0=mv[:, 0:1], scalar1=-1.0,
                                scalar2=None, op0=OP.mult)
        attn_n = ph1.tile([TQ, P], F32)
        nc.vector.tensor_scalar(out=attn_n[:], in0=outatt[:], scalar1=negmean[:],
                                scalar2=rstd_a[:], op0=OP.add, op1=OP.mult)
        attn_g = ph1.tile([TQ, P], F32)
        nc.vector.tensor_scalar(out=attn_g[:], in0=attn_n[:], scalar1=one_m_p[:],
                                scalar2=None, op0=OP.mult)
        ag_ps = ps_a.tile([P, TQ], F32, space="PSUM", tag="a_kp")
        nc.tensor.transpose(out=ag_ps[:], in_=attn_g[:], identity=ident[0:TQ, 0:TQ])
        nc.vector.tensor_copy(out=pack[:, 0:TQ], in_=ag_ps[:])
        nc.vector.tensor_copy(out=pack[0:TQ, 72:73], in_=p_q[:])
        nc.vector.tensor_scalar(out=pack[TQ:P, 72:73], in0=p_q[:], scalar1=0.0,
                                scalar2=None, op0=OP.mult)

        # ---------------- phase B: logits + partial stats ----------------
        phb = ctx.enter_context(tc.tile_pool(name="phb", bufs=2))
        for tt in range(NT):
            st_all = phb.tile([P, NE * 6], F32, tag="st")
            for ec in range(NE):
                mm = ps_b.tile([P, EC], F32, space="PSUM", tag="mm")
                esl = slice(ec * EC, (ec + 1) * EC)
                for d in range(DC):
                    nc.tensor.matmul(
                        mm[:], _r(tgtT[d][:, tt * P:(tt + 1) * P]),
                        _r(wt[d][:, esl]),
                        start=(d == 0), stop=(d == DC - 1 and not has_bout))
                if has_bout:
                    nc.tensor.matmul(mm[:], _r(ones1p[:]), _r(bvec[:, esl]),
                                     start=False, stop=True)
                nc.vector.bn_stats(out=st_all[:, ec * 6:(ec + 1) * 6], in_=mm[:])
                nc.scalar.copy(out=logits[tt][:, esl], in_=mm[:])
            mvb = phb.tile([P, 2], F32, tag="mvb")
            nc.vector.bn_aggr(out=mvb[:], in_=st_all[:])
            nc.vector.tensor_scalar(out=pack[:, 64 + tt:65 + tt], in0=mvb[:, 0:1],
                                    scalar1=float(VS), scalar2=None, op0=OP.mult)
            s2t = phb.tile([P, 1], F32, tag="s2t")
            nc.vector.tensor_tensor(out=s2t[:], in0=mvb[:, 0:1], in1=mvb[:, 0:1],
                                    op=OP.mult)
            s2u = phb.tile([P, 1], F32, tag="s2u")
            nc.vector.tensor_tensor(out=s2u[:], in0=s2t[:], in1=mvb[:, 1:2],
                                    op=OP.add)
            nc.vector.tensor_scalar(out=pack[:, 68 + tt:69 + tt], in0=s2u[:],
                                    scalar1=float(VS), scalar2=None, op0=OP.mult)

        # ---------------- collective ----------------
        if _STAGE == 1:
            for tt in range(NT):
                nc.sync.dma_start(out=out[tt * P:(tt + 1) * P, :],
                                  in_=logits[tt][:])
            return
        nc.sync.dma_start(out=cc_in[:], in_=pack[:])
        nc.gpsimd.collective_compute(
            "AllGather", OP.bypass, replica_groups=[list(range(NCORE))],
            ins=[cc_in[:].opt()], outs=[cc_out[:].opt()])
        nc.sync.dma_start(out=gath[:],
                          in_=cc_out[:].rearrange("(c p) f -> p c f", p=P))
        for tt in range(NT):
            for half in range(2):
                b = 2 * tt + half
                nc.sync.dma_start(
                    out=p_all[half * TQ:(half + 1) * TQ, tt:tt + 1],
                    in_=cc_out[b * P:b * P + TQ, 72:73])

        if _STAGE == 2:
            for tt in range(NT):
                nc.sync.dma_start(out=out[tt * P:(tt + 1) * P, :],
                                  in_=logits[tt][:])
            return
        S1 = sing.tile([P, NT], F32)
        nc.vector.tensor_reduce(
            out=S1[:], in_=gath[:, :, 64:68].rearrange("p c f -> p f c"),
            axis=AX.X, op=OP.add)
        S2 = sing.tile([P, NT], F32)
        nc.vector.tensor_reduce(
            out=S2[:], in_=gath[:, :, 68:72].rearrange("p c f -> p f c"),
            axis=AX.X, op=OP.add)
        meanv = sing.tile([P, NT], F32)
        nc.vector.tensor_scalar(out=meanv[:], in0=S1[:], scalar1=1.0 / V,
                                scalar2=None, op0=OP.mult)
        ex2 = sing.tile([P, NT], F32)
        nc.vector.tensor_scalar(out=ex2[:], in0=S2[:], scalar1=1.0 / V,
                                scalar2=None, op0=OP.mult)
        msq = sing.tile([P, NT], F32)
        nc.vector.tensor_tensor(out=msq[:], in0=meanv[:], in1=meanv[:], op=OP.mult)
        varv = sing.tile([P, NT], F32)
        nc.vector.tensor_tensor(out=varv[:], in0=ex2[:], in1=msq[:], op=OP.subtract)
        sqb = sing.tile([P, NT], F32)
        nc.scalar.activation(out=sqb[:], in_=varv[:], func=ACT.Sqrt, bias=eps128[:])
        rstdv = sing.tile([P, NT], F32)
        nc.vector.reciprocal(out=rstdv[:], in_=sqb[:])
        nmr0 = sing.tile([P, NT], F32)
        nc.vector.tensor_tensor(out=nmr0[:], in0=meanv[:], in1=rstdv[:], op=OP.mult)
        nmr = sing.tile([P, NT], F32)
        nc.vector.tensor_scalar(out=nmr[:], in0=nmr0[:], scalar1=-1.0,
                                scalar2=None, op0=OP.mult)

        # ---------------- phase D: scatter + combine + store ----------------
        phd = ctx.enter_context(tc.tile_pool(name="phd", bufs=2))
        iota_c = sing.tile([P, EC], F32)
        nc.gpsimd.iota(out=iota_c[:], pattern=[[1, EC]], base=0,
                       channel_multiplier=0,
                       allow_small_or_imprecise_dtypes=True)
        idx_ec = [sing.tile([TK, B], F32, tag=f"idxec{e}", name=f"idxec{e}")
                  for e in range(NE)]
        for e in range(NE):
            nc.vector.tensor_scalar(out=idx_ec[e][:], in0=idxT[:],
                                    scalar1=-float(e * EC), scalar2=None,
                                    op0=OP.add)
        for tt in range(NT):
            for ec in range(NE):
                esl = slice(ec * EC, (ec + 1) * EC)
                pds = []
                for half in range(2):
                    b = 2 * tt + half
                    sc = phd.tile([P, EC], F32R, tag="sc")
                    nc.vector.tensor_scalar(out=sc[:], in0=iota_c[:],
                                            scalar1=idx_ec[ec][:, b:b + 1],
                                            scalar2=None, op0=OP.is_equal)
                    pd = ps_d.tile([TQ, EC], F32, space="PSUM",
                                   tag=f"pd{half}", name=f"pd{half}")
                    nc.tensor.matmul(pd[:], _r(gath[:, b, 0:TQ]), _r(sc[:]),
                                     start=True, stop=True)
                    pds.append(pd)
                nt_ = phd.tile([P, EC], F32, tag="nt")
                nc.vector.tensor_scalar(out=nt_[:], in0=logits[tt][:, esl],
                                        scalar1=rstdv[:, tt:tt + 1],
                                        scalar2=nmr[:, tt:tt + 1],
                                        op0=OP.mult, op1=OP.add)
                if ec == NE - 1:
                    nc.vector.tensor_tensor(out=nt_[:, EC - P:EC],
                                            in0=nt_[:, EC - P:EC],
                                            in1=cmask_rep[:], op=OP.mult)
                ot = phd.tile([P, EC], F32, tag="ot")
                for half in range(2):
                    hs = slice(half * TQ, (half + 1) * TQ)
                    nc.vector.scalar_tensor_tensor(
                        out=ot[hs, :], in0=nt_[hs, :],
                        scalar=p_all[hs, tt:tt + 1],
                        in1=pds[half][:], op0=OP.mult, op1=OP.add)
                nc.sync.dma_start(out=out[tt * P:(tt + 1) * P, esl], in_=ot[:])


def _prep(inputs):
    tgt = np.ascontiguousarray(np.asarray(inputs["tgt_dec_out"], np.float32))
    skey = np.ascontiguousarray(np.asarray(inputs["src_key"], np.float32))
    idx = np.asarray(inputs["src_map_idx"]).astype(np.int64)
    W_out = np.asarray(inputs["W_out"], np.float32)
    b_out = np.asarray(inputs["b_out"], np.float32)
    W_attn = np.asarray(inputs["W_attn"], np.float32)
    b_attn = np.asarray(inputs["b_attn"], np.float32)
    v_w = np.asarray(inputs["v_w"], np.float32)
    W_lin = np.asarray(inputs["W_lin"], np.float32)
    b_lin = np.asarray(inputs["b_lin"], np.float32)

    has_bout = bool(np.any(b_out))
    wT_full = np.zeros((D, EXT), np.float32)
    wT_full[:, :V] = W_out.T
    b_pad = np.zeros(EXT, np.float32)
    b_pad[:V] = b_out
    tgtT = np.ascontiguousarray(tgt.reshape(B * TQ, D).T)
    wqT = np.ascontiguousarray(W_attn[:, :D].T)
    wkT = np.ascontiguousarray(W_attn[:, D:].T)

    in_maps = []
    for c in range(NCORE):
        cm = np.ones((1, P), np.float32)
        if c == NCORE - 1:
            cm[:] = 0.0
        m = {
            "wT": np.ascontiguousarray(wT_full[:, c * VS:(c + 1) * VS]),
            "tgtT": tgtT,
            "tgtTo": np.ascontiguousarray(tgtT[:, c * TQ:(c + 1) * TQ]),
            "tgto": np.ascontiguousarray(tgt[c]),
            "skey": np.ascontiguousarray(skey[c]),
            "skeyT": np.ascontiguousarray(skey[c].T),
            "wqT": wqT,
            "wkT": wkT,
            "battn": b_attn.reshape(1, D),
            "vw": v_w.reshape(1, D),
            "wlin": W_lin.reshape(1, D),
            "blin": b_lin.reshape(1, 1),
            "idx": (idx - c * VS).astype(np.float32),
            "colmask": cm,
            "onesrow": np.ones((1, P), np.float32),
        }
        if has_bout:
            m["bvec"] = np.ascontiguousarray(
                b_pad[c * VS:(c + 1) * VS].reshape(1, VS))
        in_maps.append(m)
    return in_maps, has_bout


def kernel(**inputs):
    in_maps, has_bout = _prep(inputs)
    key = ("nc", has_bout)
    if key not in _CACHE:
        _CACHE[key] = _build(has_bout)
    nc = _CACHE[key]
    res = run_bass_kernel_spmd(nc, in_maps, core_ids=list(range(NCORE)))
    full = np.concatenate(
        [res.results[c]["out"].reshape(B, TQ, VS) for c in range(NCORE)], axis=2)
    return full.astype(np.float32)

